# revision 56
# baseline (speedup 1.0000x reference)
"""Trainium2 Bass kernel for ActivationGATSingleHeadLayer (GNN message passing).

Reference computation (jax):
    e = relu(sum(z[src] * z[dst], -1))             # [E]
    alpha = segment_softmax(e, dst)                # two-pass in ref
    h = segment_sum(alpha[:, None] * z[src], dst)  # [N, D]
    out = relu(batchnorm(h))                       # training-mode stats

Strategy (8 NeuronCores):
  * Host shards edges by dst range: core c owns dst in [c*NPC, (c+1)*NPC).
    All segment reductions are core-local; the only collective is an
    AllReduce of 128 floats of BatchNorm statistics.
  * Segment softmax is collapsed to one pass:
        h[n] = sum_e w_e * z[src_e] / sum_e w_e,  w_e = exp(relu(e_e) - SHIFT)
    The constant SHIFT (=64) replaces the segment max: relu makes e >= 0 and
    e <= max ||z_i||^2 ~ chi2_64 stays far below SHIFT + 88, so exp never
    overflows and the result is mathematically identical.
  * Edges are sorted by dst and grouped into 128-node windows. Per 128-edge
    tile, one-hot membership matrices are built in bf16, split across the
    two elementwise engines to balance them:
        ohT[n, e] = relu(1 - (dstrel_e - n)^2)   (Scalar: square + relu)
        oh[e, n]  = (iota[n] == dstrel_e)        (DVE is_equal)
  * z[dst] rows are NOT gathered: they are expanded on the TensorEngine as
    psum_zd = ohT^T @ zwin in bf16 (FWL-enabled; one-hot selection of
    bf16-rounded z rows, rel-err contribution ~4e-3 vs the 2e-2 gate).
  * Aggregation also runs on the TensorEngine (HBM scatter-add races on
    duplicate indices): psum[win] += oh^T @ [w*z_src | w] in bf16.
    8 windows per phase share 2 PSUM banks (4 windows per 2KB bank at a
    128-f32 pitch); a start=True matmul clears its WHOLE bank, so the
    accumulation group is per (phase, bank): start only on the bank's
    first tile, stop on its last. Double-buffered psum pool removes the
    phase-boundary drain bubble.
  * BatchNorm partial sums (s1, s2) accumulate per phase so the 128-float
    AllReduce launches right after the last drain; the final normalize +
    relu + store runs in 4 window blocks so output DMA overlaps BN math.
  * z[src] rows are fetched with SWDGE dma_gather (f32, 256B elements).
    The Q7 descriptor generation runs on ONE core-pair selected by
    queue_num, ~8.6us per 1024 indices; rotating over all 4 queues runs the
    4 core-pairs concurrently (~2.2us effective per 1024-idx op), so the
    gathers are issued round-robin on queues 0-3 with a deep zsrc ring so
    consecutive gathers have no buffer dependency. int16 indices only
    reach 32767, so tiles are segregated into lo (src < SPLIT) / hi
    sections gathering from the two z-table halves. SWDGE ops are capped
    at 1024 indices (larger ops overflow the descriptor-ring carveout).
  * h is stored feature-major [128, D, NW] so BatchNorm stats reduce over
    contiguous memory; stats cross partitions via one matmul against ones,
    AllReduce of 128 floats, partition-broadcast back, normalize + relu.
"""

import sys

for _p in ("/opt/trn_rl_repo", "/root/.axon_site/_ro/trn_rl_repo"):
    if _p not in sys.path:
        sys.path.append(_p)

import ml_dtypes
import numpy as np

# ---------------------------------------------------------------- geometry
N_NODES = 50000
N_EDGES = 800000
D = 64
NCORES = 8

EPS = 1e-5          # BatchNorm eps (matches reference)
TINY = 1e-30        # denom guard for isolated nodes
SHIFT = 64.0        # constant subtracted inside exp
WIN = 128           # nodes per aggregation window (= PSUM partition dim)
MAX_PSUM_WIN = 8    # windows per phase; 4 windows share a 2KB PSUM bank
                    # (each window needs 65 f32 -> packed at 128-f32 pitch)
CHUNK_TILES = 8     # tiles per gather chunk; SWDGE ops above ~1024 indices
                    # overflow the descriptor-ring carveout and hang


def _derive(n_nodes, split):
    npc = n_nodes // NCORES
    nw = -(-npc // WIN)
    return dict(
        n_nodes=n_nodes,
        npc=npc,
        nw=nw,
        h_rows=nw * WIN,
        split=split,      # hi-table base: table1 = z[split : split + 32768]
        lo_max=32768,     # table0 = z[0 : 32768]
    )


CFG = _derive(N_NODES, split=N_NODES - 32768)


# ---------------------------------------------------------------- host prep
def _wrap_tile_idx(arr):
    """[T, 128] int -> [128, T, 8] int16 SWDGE layout, partition-major:
    within-tile edge p at [p%16 (+16g), tile, p//16]."""
    t = arr.shape[0]
    w = arr.reshape(t, 8, 16).transpose(0, 2, 1).astype(np.int16)  # [T,16,8]
    w = np.tile(w, (1, 8, 1))                                      # [T,128,8]
    return w.transpose(1, 0, 2).copy()                             # [128,T,8]


def prep_inputs(z, src, dst, gamma, beta, cfg=CFG):
    """Shard edges by dst range, sort by dst, build window/tile plan.

    Returns (in_maps, plan). The plan (tile metadata, section table kinds,
    chunking) is identical across cores, as SPMD requires.
    """
    z = np.ascontiguousarray(np.asarray(z, dtype=np.float32))
    src = np.asarray(src).astype(np.int64)
    dst = np.asarray(dst).astype(np.int64)
    gamma = np.asarray(gamma, dtype=np.float32)
    beta = np.asarray(beta, dtype=np.float32)

    npc, split, nw = cfg["npc"], cfg["split"], cfg["nw"]
    h_rows = cfg["h_rows"]

    # Per-core, per-window edge lists sorted by src. The z table is split as
    # table0 = z[0:32768], table1 = z[split:], overlapping on
    # [split, 32768): edges there can gather from either table, so the tile
    # counts need only ONE ceil per window (nt_tot), with the type-0/type-1
    # boundary chosen per core inside the overlap.
    lo_max = cfg["lo_max"]
    win_edges = [[None] * nw for _ in range(NCORES)]
    core_of = dst // npc
    for c in range(NCORES):
        m = core_of == c
        s, ld = src[m], dst[m] - c * npc
        order = np.argsort(ld, kind="stable")
        s, ld = s[order], ld[order]
        w_of = ld // WIN
        for w in range(nw):
            wm = w_of == w
            sg, dg = s[wm], ld[wm]
            o2 = np.argsort(sg, kind="stable")
            win_edges[c][w] = (sg[o2], dg[o2])

    # equalized tile counts (identical across cores)
    nt = np.zeros((2, nw), dtype=np.int64)
    for w in range(nw):
        tot = max(
            -(-len(win_edges[c][w][0]) // 128) for c in range(NCORES)
        )
        tot = max(tot, 1)  # every window needs >= 1 tile (PSUM init)
        # tiles that MUST be type 0 (src < split can only use table0)
        n0 = max(
            -(-int(np.sum(win_edges[c][w][0] < split)) // 128)
            for c in range(NCORES)
        )
        # a type-0 tile must stay within table0: feasible while every core
        # can fill n0*128 slots with src < lo_max edges (always true here
        # since lo_max covers 65% of nodes); cap n0 at tot.
        n0 = min(n0, tot)
        nt[0, w] = n0
        nt[1, w] = tot - n0

    # phases / sections / global tile order
    phases = [list(range(i, min(i + MAX_PSUM_WIN, nw)))
              for i in range(0, nw, MAX_PSUM_WIN)]
    sections = []   # (ty, [(w, local_tile_j), ...]) in global tile order
    tile_meta = []  # (window, start, stop) per global tile
    # start/stop are PSUM-accumulation-group flags. A matmul with start=True
    # clears the whole 2KB PSUM bank, and 4 windows share a bank (128-f32
    # pitch), so the group is per (phase, bank): start on the bank's first
    # tile in global order, stop on its last.
    for ph in phases:
        w0 = ph[0]
        ordered = []  # (ty, w, j) in global order within the phase
        ph_sections = []
        for ty in range(2):
            tl = []
            for w in ph:
                for j in range(nt[ty, w]):
                    ordered.append((ty, w, j))
                    tl.append((w, j))
            if tl:
                ph_sections.append((ty, tl))
        first_of_bank = {}
        last_of_bank = {}
        for k, (ty, w, j) in enumerate(ordered):
            bank = (w - w0) // 4
            if bank not in first_of_bank:
                first_of_bank[bank] = k
            last_of_bank[bank] = k
        for k, (ty, w, j) in enumerate(ordered):
            bank = (w - w0) // 4
            tile_meta.append(
                (w, k == first_of_bank[bank], k == last_of_bank[bank])
            )
        sections.extend(ph_sections)

    t_total = len(tile_meta)
    plan = dict(
        cfg=cfg,
        nt=nt,
        phases=phases,
        sections=sections,
        tile_meta=tile_meta,
        t_total=t_total,
    )

    gb = np.stack([gamma, beta]).astype(np.float32)
    iota = np.arange(128, dtype=np.float32)
    niota = -iota
    # iotat_rep[p, t, n] = n (device constant for the edge-major one-hot
    # is_equal build; shipped from host so no DVE copies are needed, since
    # perf-mode DVE copies lock GPSIMD out of the shared SBUF port)
    iotat_rep = np.broadcast_to(
        iota.astype(ml_dtypes.bfloat16), (128, CHUNK_TILES, 128)
    ).copy()

    in_maps = []
    for c in range(NCORES):
        # per-window split of the src-sorted edges into type0/type1 slices
        ed = [[None, None] for _ in range(nw)]
        for w in range(nw):
            sg, dg = win_edges[c][w]
            k0 = int(np.searchsorted(sg, lo_max))
            k0 = min(k0, int(nt[0, w]) * 128)
            assert len(sg) - k0 <= int(nt[1, w]) * 128, (c, w)
            if k0 < len(sg):
                assert sg[k0] >= split, (c, w)
            ed[w][0] = (sg[:k0], dg[:k0])
            ed[w][1] = (sg[k0:] - split, dg[k0:])

        isrc = np.zeros((t_total, 128), dtype=np.int64)
        drel = np.full((t_total, 128), -1.0, dtype=np.float32)
        g = 0
        for ty, tl in sections:
            for w, j in tl:
                s, ld = ed[w][ty]
                seg_s = s[j * 128 : (j + 1) * 128]
                seg_d = ld[j * 128 : (j + 1) * 128]
                k = len(seg_s)
                isrc[g, :k] = seg_s
                drel[g, :k] = (seg_d - w * WIN).astype(np.float32)
                g += 1
        assert g == t_total

        zs = np.zeros((h_rows, D), dtype=ml_dtypes.bfloat16)
        zs[:npc] = z[c * npc : (c + 1) * npc].astype(ml_dtypes.bfloat16)

        in_maps.append(
            {
                "z": z,
                "zs": zs,
                "isrc": _wrap_tile_idx(isrc),
                "drel": drel.T.astype(ml_dtypes.bfloat16),  # [128, T] edge-major
                "drelf": drel.reshape(-1).astype(ml_dtypes.bfloat16),
                "gb": gb,
                "iota": iota,
                "niota": niota,
                "iotat_rep": iotat_rep,
            }
        )
    return in_maps, plan


# ---------------------------------------------------------------- device graph
def build_nc(plan, n_total_nodes=None):
    """Build the SPMD Bass graph (identical on all cores)."""
    from concourse import bacc, tile
    from concourse.bass import mybir

    f32 = mybir.dt.float32
    bf16 = mybir.dt.bfloat16
    i16 = mybir.dt.int16
    AX = mybir.AxisListType
    ALU = mybir.AluOpType
    ACTF = mybir.ActivationFunctionType

    cfg = plan["cfg"]
    nw, split, lo_max = cfg["nw"], cfg["split"], cfg["lo_max"]
    h_rows, n_nodes = cfg["h_rows"], cfg["n_nodes"]
    if n_total_nodes is None:
        n_total_nodes = n_nodes
    t_total = plan["t_total"]
    tile_meta = plan["tile_meta"]

    nc = bacc.Bacc(
        "TRN2",
        target_bir_lowering=False,
        debug=False,
        num_devices=NCORES,
        num_swdge_queues=4,
    )

    z_d = nc.dram_tensor("z", [n_nodes, D], f32, kind="ExternalInput")
    zs_d = nc.dram_tensor("zs", [h_rows, D], bf16, kind="ExternalInput")
    isrc_d = nc.dram_tensor("isrc", [128, t_total, 8], i16, kind="ExternalInput")
    drel_d = nc.dram_tensor("drel", [128, t_total], bf16, kind="ExternalInput")
    drelf_d = nc.dram_tensor("drelf", [t_total * 128], bf16, kind="ExternalInput")
    gb_d = nc.dram_tensor("gb", [2, D], f32, kind="ExternalInput")
    iota_d = nc.dram_tensor("iota", [128], f32, kind="ExternalInput")
    niota_d = nc.dram_tensor("niota", [128], f32, kind="ExternalInput")
    iotat_rep_d = nc.dram_tensor(
        "iotat_rep", [128, CHUNK_TILES, 128], bf16, kind="ExternalInput"
    )
    out_d = nc.dram_tensor("out", [h_rows, D], f32, kind="ExternalOutput")

    CT = CHUNK_TILES

    with tile.TileContext(nc) as tc:
        with (
            tc.tile_pool(name="const", bufs=1) as constp,
            tc.tile_pool(name="data", bufs=10) as datap,
            tc.tile_pool(name="oh", bufs=9) as ohp,
            tc.tile_pool(name="small", bufs=10) as smallp,
            tc.tile_pool(name="fin", bufs=1) as finp,
            tc.tile_pool(name="yblk", bufs=2) as yblkp,
            tc.tile_pool(name="dram", bufs=1, space="DRAM") as dramp,
            tc.tile_pool(name="psum", bufs=2, space="PSUM") as psump,
            tc.tile_pool(name="psz", bufs=4, space="PSUM") as pszp,
        ):
            tinyb = constp.tile([128, 1], f32)
            nc.vector.memset(tinyb[:], TINY)
            shiftb = constp.tile([128, 1], f32)
            nc.vector.memset(shiftb[:], -SHIFT)
            epsb = constp.tile([128, 1], f32)
            nc.vector.memset(epsb[:], EPS)
            ones = constp.tile([128, 1], f32)
            nc.vector.memset(ones[:], 1.0)
            # preload all SWDGE indices + dstrel (partition-major layouts);
            # sliced so early chunks' indices land before the whole preload.
            isrc_sb = constp.tile([128, t_total, 8], i16)
            qt = -(-t_total // 4)
            for _q in range(4):
                lo_t, hi_t = _q * qt, min((_q + 1) * qt, t_total)
                nc.sync.dma_start(
                    isrc_sb[:, lo_t:hi_t, :], isrc_d[:, lo_t:hi_t, :]
                )
            drel_sb = constp.tile([128, t_total], bf16)
            nc.sync.dma_start(drel_sb[:], drel_d[:, :])

            # iotat_rep[p, t, n] = n (host constant)
            iotat_rep = constp.tile([128, CHUNK_TILES, 128], bf16)
            nc.sync.dma_start(iotat_rep[:], iotat_rep_d[:, :, :])
            # niotac[p, 0] = -p, bias for the Scalar-engine ohT build
            niotac = constp.tile([128, 1], f32)
            nc.sync.dma_start(niotac[:], niota_d.ap().unsqueeze(1))
            onesb = constp.tile([128, 1], f32)
            nc.vector.memset(onesb[:], 1.0)

            h_all = finp.tile([128, D, nw], f32)
            # running BatchNorm stats, accumulated per phase directly into
            # the contiguous [s1 | s2] layout the stats matmul consumes
            stats = finp.tile([128, 2 * D], f32, tag="stats")
            nc.vector.memset(stats[:], 0.0)
            s1acc = stats[:, 0:D]
            s2acc = stats[:, D : 2 * D]

            g = 0  # global tile cursor
            kq = 0  # chunk counter for SWDGE queue rotation
            for ph in plan["phases"]:
                nwp = len(ph)
                w0 = ph[0]
                psb = psump.tile([128, MAX_PSUM_WIN, 128], f32, tag="psb")
                zwin = datap.tile([128, MAX_PSUM_WIN, D], bf16, tag="zwin")
                nc.sync.dma_start(
                    zwin[:, 0:nwp, :],
                    zs_d[w0 * WIN : (w0 + nwp) * WIN, :].rearrange(
                        "(w p) d -> p w d", p=128
                    ),
                )
                ph_secs = [
                    (ty, tl) for (ty, tl) in plan["sections"] if tl[0][0] in ph
                ]
                for ty, tl in ph_secs:
                    table = z_d[0:lo_max, :] if ty == 0 else z_d[split:n_nodes, :]
                    for c0 in range(0, len(tl), CT):
                        ct = min(CT, len(tl) - c0)
                        t0 = g + c0
                        ne = ct * 128

                        zsrc = datap.tile([128, CT, D], f32, tag="zsrc")
                        for g0 in range(0, ct, 8):
                            gc = min(8, ct - g0)
                            nc.gpsimd.dma_gather(
                                zsrc[:, g0 : g0 + gc, :],
                                table,
                                isrc_sb[:, t0 + g0 : t0 + g0 + gc, :],
                                gc * 128,
                                gc * 128,
                                D,
                                queue_num=kq % 4,
                            )
                            kq += 1

                        # node-major one-hot: ohT[n, e] = (drel_e == n).
                        # A/B by chunk parity: even -> Scalar square+relu,
                        # odd -> DVE is_equal with both operands contiguous
                        # (tests whether the 2x DVE mode engages).
                        drbc = ohp.tile([128, CT * 128], bf16, tag="drbc")
                        nc.sync.dma_start(
                            drbc[:, 0:ne],
                            drelf_d[t0 * 128 : t0 * 128 + ne].partition_broadcast(
                                128
                            ),
                        )
                        ohT = ohp.tile([128, CT * 128], bf16, tag="ohT")
                        nc.scalar.activation(
                            ohT[:, 0:ne], drbc[:, 0:ne], ACTF.Square,
                            bias=niotac[:], scale=1.0,
                        )
                        nc.scalar.activation(
                            ohT[:, 0:ne], ohT[:, 0:ne], ACTF.Relu,
                            bias=onesb[:], scale=-1.0,
                        )

                        # z[dst] expansion: psum_zd[:, t, :] = ohT_t^T @ zwin_t
                        # (pzd spans 2 PSUM banks at CT=16; a start=True
                        # matmul clears its whole bank, so open/close one
                        # accumulation group per 8-tile bank)
                        pzd = pszp.tile([128, CT, D], f32, tag="zd")
                        for tl_i in range(ct):
                            win = tile_meta[t0 + tl_i][0]
                            nc.tensor.matmul(
                                pzd[:, tl_i, :],
                                ohT[:, tl_i * 128 : (tl_i + 1) * 128],
                                zwin[:, win - w0, :],
                                start=tl_i % 8 == 0,
                                stop=tl_i % 8 == 7 or tl_i == ct - 1,
                            )

                        # edge scores and weights (prod in bf16 so the
                        # reduce reads 2 elems/cycle)
                        prod = datap.tile([128, CT, D], bf16, tag="prod")
                        e = smallp.tile([128, CT], f32, tag="e")
                        wt = smallp.tile([128, CT], f32, tag="wt")
                        nc.vector.tensor_mul(
                            prod[:, 0:ct, :], zsrc[:, 0:ct, :], pzd[:, 0:ct, :]
                        )
                        nc.vector.tensor_reduce(
                            e[:, 0:ct], prod[:, 0:ct, :], axis=AX.X, op=ALU.add
                        )
                        nc.scalar.activation(
                            e[:, 0:ct], e[:, 0:ct], ACTF.Relu,
                            bias=0.0, scale=1.0,
                        )
                        nc.scalar.activation(
                            wt[:, 0:ct], e[:, 0:ct], ACTF.Exp,
                            bias=shiftb[:], scale=1.0,
                        )

                        # vals = [w * z_src | w] in bf16
                        vals = datap.tile([128, CT, D + 1], bf16, tag="vals")
                        nc.scalar.copy(vals[:, 0:ct, D], wt[:, 0:ct])
                        nc.vector.tensor_mul(
                            vals[:, 0:ct, 0:D],
                            zsrc[:, 0:ct, :],
                            wt[:, 0:ct].unsqueeze(2).broadcast_to((128, ct, D)),
                        )

                        # aggregation one-hot (edge-major) in bf16
                        oh = ohp.tile([128, CT, 128], bf16, tag="oh")
                        nc.vector.tensor_tensor(
                            oh[:, 0:ct, :],
                            iotat_rep[:, 0:ct, :],
                            drel_sb[:, t0 : t0 + ct]
                            .unsqueeze(2)
                            .broadcast_to((128, ct, 128)),
                            op=ALU.is_equal,
                        )

                        for tl_i in range(ct):
                            win, st, sp = tile_meta[t0 + tl_i]
                            slot = win - w0
                            nc.tensor.matmul(
                                psb[:, slot, 0 : D + 1],
                                oh[:, tl_i, :],
                                vals[:, tl_i, :],
                                start=st,
                                stop=sp,
                            )
                    g += len(tl)

                # drain phase: h = num / denom, written feature-major
                denp = smallp.tile([128, MAX_PSUM_WIN], f32, tag="den")
                recp = smallp.tile([128, MAX_PSUM_WIN], f32, tag="rec")
                nc.scalar.activation(
                    denp[:, 0:nwp], psb[:, 0:nwp, D], ACTF.Identity,
                    bias=tinyb[:], scale=1.0,
                )
                nc.vector.reciprocal(recp[:, 0:nwp], denp[:, 0:nwp])
                # h = num/denom via per-window Scalar scale-copies (ScalarE
                # reads PSUM fast and is less loaded than the DVE pacer)
                for wv in range(nwp):
                    nc.scalar.activation(
                        h_all[:, :, w0 + wv],
                        psb[:, wv, 0:D],
                        ACTF.Copy,
                        bias=0.0,
                        scale=recp[:, wv : wv + 1],
                    )

                # accumulate BatchNorm partial stats for this phase so the
                # AllReduce can start right after the last drain
                hsqp = smallp.tile([128, D, MAX_PSUM_WIN], f32, tag="hsqp")
                nc.scalar.square(
                    hsqp[:, :, 0:nwp], h_all[:, :, w0 : w0 + nwp]
                )
                s1t = smallp.tile([128, D], f32, tag="s1t")
                s2t = smallp.tile([128, D], f32, tag="s2t")
                nc.vector.tensor_reduce(
                    s1t[:], h_all[:, :, w0 : w0 + nwp], axis=AX.X, op=ALU.add
                )
                nc.vector.tensor_add(s1acc, s1acc, s1t[:])
                nc.vector.tensor_reduce(
                    s2t[:], hsqp[:, :, 0:nwp], axis=AX.X, op=ALU.add
                )
                nc.vector.tensor_add(s2acc, s2acc, s2t[:])

            # ---- BatchNorm stats: s1 = sum(h), s2 = sum(h^2) over all nodes
            ps = pszp.tile([1, 2 * D], f32, tag="zd")
            nc.tensor.matmul(ps[:], ones[:], stats[:], start=True, stop=True)
            srow = smallp.tile([1, 2 * D], f32, tag="srow")
            nc.scalar.copy(srow[:], ps[:])

            cc_in = dramp.tile([1, 2 * D], f32)
            cc_out = dramp.tile([1, 2 * D], f32)
            nc.sync.dma_start(cc_in[:], srow[:])
            nc.gpsimd.collective_compute(
                "AllReduce",
                ALU.add,
                ins=[cc_in.opt()],
                outs=[cc_out.opt()],
                replica_groups=[list(range(NCORES))],
            )

            G = smallp.tile([128, 2 * D], f32, tag="G")
            nc.sync.dma_start(G[:], cc_out[:].squeeze(0).partition_broadcast(128))
            gbB = constp.tile([128, 2 * D], f32)
            nc.sync.dma_start(gbB[:], gb_d.ap().flatten().partition_broadcast(128))

            inv_n = 1.0 / float(n_total_nodes)
            mean = smallp.tile([128, D], f32, tag="mean")
            var = smallp.tile([128, D], f32, tag="var")
            nc.scalar.mul(mean[:], G[:, 0:D], inv_n)
            nc.scalar.mul(var[:], G[:, D : 2 * D], inv_n)
            msq = smallp.tile([128, D], f32, tag="msq")
            nc.vector.tensor_mul(msq[:], mean[:], mean[:])
            nc.vector.tensor_sub(var[:], var[:], msq[:])
            std = smallp.tile([128, D], f32, tag="std")
            nc.scalar.activation(std[:], var[:], ACTF.Sqrt, bias=epsb[:], scale=1.0)
            rstd = smallp.tile([128, D], f32, tag="rstd")
            nc.vector.reciprocal(rstd[:], std[:])

            a = smallp.tile([128, D], f32, tag="a")
            b = smallp.tile([128, D], f32, tag="b")
            nc.vector.tensor_mul(a[:], gbB[:, 0:D], rstd[:])
            nc.vector.tensor_mul(b[:], mean[:], a[:])
            nc.vector.tensor_sub(b[:], gbB[:, D : 2 * D], b[:])

            # y stored node-major so the output DMA gets 256B-contiguous
            # runs; built and stored in window blocks so the output DMA
            # overlaps the BN math of later blocks.
            outv = out_d.ap().rearrange("(c p) f -> p c f", p=128)
            nblk = 6
            bw = -(-nw // nblk)
            for blk in range(nblk):
                wl, wh = blk * bw, min((blk + 1) * bw, nw)
                if wl >= wh:
                    break
                y = yblkp.tile([128, bw, D], f32, tag="y")
                nc.vector.tensor_mul(
                    y[:, 0 : wh - wl, :],
                    h_all[:, :, wl:wh].transpose((0, 2, 1)),
                    a[:].unsqueeze(1).broadcast_to((128, wh - wl, D)),
                )
                nc.vector.tensor_add(
                    y[:, 0 : wh - wl, :],
                    y[:, 0 : wh - wl, :],
                    b[:].unsqueeze(1).broadcast_to((128, wh - wl, D)),
                )
                nc.vector.tensor_relu(
                    y[:, 0 : wh - wl, :], y[:, 0 : wh - wl, :]
                )
                nc.sync.dma_start(outv[:, wl:wh, :], y[:, 0 : wh - wl, :])

    nc.compile()
    return nc


# ---------------------------------------------------------------- entry point
TRACE = False          # set True by test harnesses to capture exec_time_ns
LAST_RESULT = None     # BassKernelResults of the most recent kernel() call


def kernel(**inputs):
    z = inputs["z"]
    src = inputs["src"]
    dst = inputs["dst"]
    gamma = inputs["gamma"]
    beta = inputs["beta"]

    from concourse.bass_utils import run_bass_kernel_spmd

    in_maps, plan = prep_inputs(z, src, dst, gamma, beta)
    nc = build_nc(plan)
    res = run_bass_kernel_spmd(
        nc, in_maps, core_ids=list(range(NCORES)), trace=TRACE
    )
    global LAST_RESULT
    LAST_RESULT = res

    npc = CFG["npc"]
    out = np.empty((N_NODES, D), dtype=np.float32)
    for c in range(NCORES):
        out[c * npc : (c + 1) * npc] = res.results[c]["out"][:npc]
    return out


# revision 57
# speedup vs baseline: 1.1412x; 1.1412x over previous
"""Trainium2 Bass kernel for ActivationGATSingleHeadLayer (GNN message passing).

Reference computation (jax):
    e = relu(sum(z[src] * z[dst], -1))             # [E]
    alpha = segment_softmax(e, dst)                # two-pass in ref
    h = segment_sum(alpha[:, None] * z[src], dst)  # [N, D]
    out = relu(batchnorm(h))                       # training-mode stats

Strategy (8 NeuronCores):
  * Host shards edges by dst range: core c owns dst in [c*NPC, (c+1)*NPC).
    All segment reductions are core-local; the only collective is an
    AllReduce of 128 floats of BatchNorm statistics.
  * Segment softmax is collapsed to one pass:
        h[n] = sum_e w_e * z[src_e] / sum_e w_e,  w_e = exp(relu(e_e) - SHIFT)
    The constant SHIFT (=64) replaces the segment max: relu makes e >= 0 and
    e <= max ||z_i||^2 ~ chi2_64 stays far below SHIFT + 88, so exp never
    overflows and the result is mathematically identical.
  * Edges are sorted by dst and grouped into 128-node windows. Per 128-edge
    tile, one-hot membership matrices are built in bf16, split across the
    two elementwise engines to balance them:
        ohT[n, e] = relu(1 - (dstrel_e - n)^2)   (Scalar: square + relu)
        oh[e, n]  = (iota[n] == dstrel_e)        (DVE is_equal)
  * z[dst] rows are NOT gathered: they are expanded on the TensorEngine as
    psum_zd = ohT^T @ zwin in bf16 (FWL-enabled; one-hot selection of
    bf16-rounded z rows, rel-err contribution ~4e-3 vs the 2e-2 gate).
  * Aggregation also runs on the TensorEngine (HBM scatter-add races on
    duplicate indices): psum[win] += oh^T @ [w*z_src | w] in bf16.
    8 windows per phase share 2 PSUM banks (4 windows per 2KB bank at a
    128-f32 pitch); a start=True matmul clears its WHOLE bank, so the
    accumulation group is per (phase, bank): start only on the bank's
    first tile, stop on its last. Double-buffered psum pool removes the
    phase-boundary drain bubble.
  * BatchNorm partial sums (s1, s2) accumulate per phase so the 128-float
    AllReduce launches right after the last drain; the final normalize +
    relu + store runs in 4 window blocks so output DMA overlaps BN math.
  * z[src] rows are fetched with SWDGE dma_gather (f32, 256B elements).
    The Q7 descriptor generation runs on ONE core-pair selected by
    queue_num, ~8.6us per 1024 indices; rotating over all 4 queues runs the
    4 core-pairs concurrently (~2.2us effective per 1024-idx op), so the
    gathers are issued round-robin on queues 0-3 with a deep zsrc ring so
    consecutive gathers have no buffer dependency. int16 indices only
    reach 32767, so tiles are segregated into lo (src < SPLIT) / hi
    sections gathering from the two z-table halves. SWDGE ops are capped
    at 1024 indices (larger ops overflow the descriptor-ring carveout).
  * h is stored feature-major [128, D, NW] so BatchNorm stats reduce over
    contiguous memory; stats cross partitions via one matmul against ones,
    AllReduce of 128 floats, partition-broadcast back, normalize + relu.
"""

import sys

for _p in ("/opt/trn_rl_repo", "/root/.axon_site/_ro/trn_rl_repo"):
    if _p not in sys.path:
        sys.path.append(_p)

import ml_dtypes
import numpy as np

# ---------------------------------------------------------------- geometry
N_NODES = 50000
N_EDGES = 800000
D = 64
NCORES = 8

EPS = 1e-5          # BatchNorm eps (matches reference)
TINY = 1e-30        # denom guard for isolated nodes
SHIFT = 64.0        # constant subtracted inside exp
WIN = 128           # nodes per aggregation window (= PSUM partition dim)
MAX_PSUM_WIN = 8    # windows per phase; 4 windows share a 2KB PSUM bank
                    # (each window needs 65 f32 -> packed at 128-f32 pitch)
CHUNK_TILES = 8     # tiles per gather chunk; SWDGE ops above ~1024 indices
                    # overflow the descriptor-ring carveout and hang


def _derive(n_nodes, split):
    npc = n_nodes // NCORES
    nw = -(-npc // WIN)
    return dict(
        n_nodes=n_nodes,
        npc=npc,
        nw=nw,
        h_rows=nw * WIN,
        split=split,      # hi-table base: table1 = z[split : split + 32768]
        lo_max=32768,     # table0 = z[0 : 32768]
    )


CFG = _derive(N_NODES, split=N_NODES - 32768)


# ---------------------------------------------------------------- host prep
def _wrap_tile_idx(arr):
    """[T, 128] int -> [128, T, 8] int16 SWDGE layout, partition-major:
    within-tile edge p at [p%16 (+16g), tile, p//16]."""
    t = arr.shape[0]
    w = arr.reshape(t, 8, 16).transpose(0, 2, 1).astype(np.int16)  # [T,16,8]
    w = np.tile(w, (1, 8, 1))                                      # [T,128,8]
    return w.transpose(1, 0, 2).copy()                             # [128,T,8]


def prep_inputs(z, src, dst, gamma, beta, cfg=CFG):
    """Shard edges by dst range, sort by dst, build window/tile plan.

    Returns (in_maps, plan). The plan (tile metadata, section table kinds,
    chunking) is identical across cores, as SPMD requires.
    """
    z = np.ascontiguousarray(np.asarray(z, dtype=np.float32))
    src = np.asarray(src).astype(np.int64)
    dst = np.asarray(dst).astype(np.int64)
    gamma = np.asarray(gamma, dtype=np.float32)
    beta = np.asarray(beta, dtype=np.float32)

    npc, split, nw = cfg["npc"], cfg["split"], cfg["nw"]
    h_rows = cfg["h_rows"]

    # Per-core, per-window edge lists sorted by src. The z table is split as
    # table0 = z[0:32768], table1 = z[split:], overlapping on
    # [split, 32768): edges there can gather from either table, so the tile
    # counts need only ONE ceil per window (nt_tot), with the type-0/type-1
    # boundary chosen per core inside the overlap.
    lo_max = cfg["lo_max"]
    win_edges = [[None] * nw for _ in range(NCORES)]
    core_of = dst // npc
    for c in range(NCORES):
        m = core_of == c
        s, ld = src[m], dst[m] - c * npc
        order = np.argsort(ld, kind="stable")
        s, ld = s[order], ld[order]
        w_of = ld // WIN
        for w in range(nw):
            wm = w_of == w
            sg, dg = s[wm], ld[wm]
            o2 = np.argsort(sg, kind="stable")
            win_edges[c][w] = (sg[o2], dg[o2])

    # equalized tile counts (identical across cores)
    nt = np.zeros((2, nw), dtype=np.int64)
    for w in range(nw):
        tot = max(
            -(-len(win_edges[c][w][0]) // 128) for c in range(NCORES)
        )
        tot = max(tot, 1)  # every window needs >= 1 tile (PSUM init)
        # tiles that MUST be type 0 (src < split can only use table0)
        n0 = max(
            -(-int(np.sum(win_edges[c][w][0] < split)) // 128)
            for c in range(NCORES)
        )
        # a type-0 tile must stay within table0: feasible while every core
        # can fill n0*128 slots with src < lo_max edges (always true here
        # since lo_max covers 65% of nodes); cap n0 at tot.
        n0 = min(n0, tot)
        nt[0, w] = n0
        nt[1, w] = tot - n0

    # phases / sections / global tile order
    phases = [list(range(i, min(i + MAX_PSUM_WIN, nw)))
              for i in range(0, nw, MAX_PSUM_WIN)]
    sections = []   # (ty, [(w, local_tile_j), ...]) in global tile order
    tile_meta = []  # (window, start, stop) per global tile
    # start/stop are PSUM-accumulation-group flags. A matmul with start=True
    # clears the whole 2KB PSUM bank, and 4 windows share a bank (128-f32
    # pitch), so the group is per (phase, bank): start on the bank's first
    # tile in global order, stop on its last.
    for ph in phases:
        w0 = ph[0]
        ordered = []  # (ty, w, j) in global order within the phase
        ph_sections = []
        for ty in range(2):
            tl = []
            for w in ph:
                for j in range(nt[ty, w]):
                    ordered.append((ty, w, j))
                    tl.append((w, j))
            if tl:
                ph_sections.append((ty, tl))
        first_of_bank = {}
        last_of_bank = {}
        for k, (ty, w, j) in enumerate(ordered):
            bank = (w - w0) // 4
            if bank not in first_of_bank:
                first_of_bank[bank] = k
            last_of_bank[bank] = k
        for k, (ty, w, j) in enumerate(ordered):
            bank = (w - w0) // 4
            tile_meta.append(
                (w, k == first_of_bank[bank], k == last_of_bank[bank])
            )
        sections.extend(ph_sections)

    t_total = len(tile_meta)
    plan = dict(
        cfg=cfg,
        nt=nt,
        phases=phases,
        sections=sections,
        tile_meta=tile_meta,
        t_total=t_total,
    )

    gb = np.stack([gamma, beta]).astype(np.float32)
    iota = np.arange(128, dtype=np.float32)
    niota = -iota
    # iotat_rep[p, t, n] = n (device constant for the edge-major one-hot
    # is_equal build; shipped from host so no DVE copies are needed, since
    # perf-mode DVE copies lock GPSIMD out of the shared SBUF port)
    iotat_rep = np.broadcast_to(
        iota.astype(ml_dtypes.bfloat16), (128, CHUNK_TILES, 128)
    ).copy()

    in_maps = []
    for c in range(NCORES):
        # per-window split of the src-sorted edges into type0/type1 slices
        ed = [[None, None] for _ in range(nw)]
        for w in range(nw):
            sg, dg = win_edges[c][w]
            k0 = int(np.searchsorted(sg, lo_max))
            k0 = min(k0, int(nt[0, w]) * 128)
            assert len(sg) - k0 <= int(nt[1, w]) * 128, (c, w)
            if k0 < len(sg):
                assert sg[k0] >= split, (c, w)
            ed[w][0] = (sg[:k0], dg[:k0])
            ed[w][1] = (sg[k0:] - split, dg[k0:])

        isrc = np.zeros((t_total, 128), dtype=np.int64)
        drel = np.full((t_total, 128), -1.0, dtype=np.float32)
        g = 0
        for ty, tl in sections:
            for w, j in tl:
                s, ld = ed[w][ty]
                seg_s = s[j * 128 : (j + 1) * 128]
                seg_d = ld[j * 128 : (j + 1) * 128]
                k = len(seg_s)
                isrc[g, :k] = seg_s
                drel[g, :k] = (seg_d - w * WIN).astype(np.float32)
                g += 1
        assert g == t_total

        zs = np.zeros((h_rows, D), dtype=ml_dtypes.bfloat16)
        zs[:npc] = z[c * npc : (c + 1) * npc].astype(ml_dtypes.bfloat16)

        in_maps.append(
            {
                "z": z,
                "zs": zs,
                "isrc": _wrap_tile_idx(isrc),
                "drel": drel.T.astype(ml_dtypes.bfloat16),  # [128, T] edge-major
                "drelf": drel.reshape(-1).astype(ml_dtypes.bfloat16),
                "gb": gb,
                "iota": iota,
                "niota": niota,
                "iotat_rep": iotat_rep,
            }
        )
    return in_maps, plan


# ---------------------------------------------------------------- device graph
def build_nc(plan, n_total_nodes=None):
    """Build the SPMD Bass graph (identical on all cores)."""
    from concourse import bacc, tile
    from concourse.bass import mybir

    f32 = mybir.dt.float32
    bf16 = mybir.dt.bfloat16
    i16 = mybir.dt.int16
    AX = mybir.AxisListType
    ALU = mybir.AluOpType
    ACTF = mybir.ActivationFunctionType

    cfg = plan["cfg"]
    nw, split, lo_max = cfg["nw"], cfg["split"], cfg["lo_max"]
    h_rows, n_nodes = cfg["h_rows"], cfg["n_nodes"]
    if n_total_nodes is None:
        n_total_nodes = n_nodes
    t_total = plan["t_total"]
    tile_meta = plan["tile_meta"]

    nc = bacc.Bacc(
        "TRN2",
        target_bir_lowering=False,
        debug=False,
        num_devices=NCORES,
        num_swdge_queues=4,
    )

    z_d = nc.dram_tensor("z", [n_nodes, D], f32, kind="ExternalInput")
    zs_d = nc.dram_tensor("zs", [h_rows, D], bf16, kind="ExternalInput")
    isrc_d = nc.dram_tensor("isrc", [128, t_total, 8], i16, kind="ExternalInput")
    drel_d = nc.dram_tensor("drel", [128, t_total], bf16, kind="ExternalInput")
    drelf_d = nc.dram_tensor("drelf", [t_total * 128], bf16, kind="ExternalInput")
    gb_d = nc.dram_tensor("gb", [2, D], f32, kind="ExternalInput")
    iota_d = nc.dram_tensor("iota", [128], f32, kind="ExternalInput")
    niota_d = nc.dram_tensor("niota", [128], f32, kind="ExternalInput")
    iotat_rep_d = nc.dram_tensor(
        "iotat_rep", [128, CHUNK_TILES, 128], bf16, kind="ExternalInput"
    )
    out_d = nc.dram_tensor("out", [h_rows, D], f32, kind="ExternalOutput")

    CT = CHUNK_TILES

    with tile.TileContext(nc) as tc:
        with (
            tc.tile_pool(name="const", bufs=1) as constp,
            tc.tile_pool(name="data", bufs=10) as datap,
            tc.tile_pool(name="oh", bufs=9) as ohp,
            tc.tile_pool(name="small", bufs=10) as smallp,
            tc.tile_pool(name="fin", bufs=1) as finp,
            tc.tile_pool(name="yblk", bufs=2) as yblkp,
            tc.tile_pool(name="dram", bufs=1, space="DRAM") as dramp,
            tc.tile_pool(name="psum", bufs=2, space="PSUM") as psump,
            tc.tile_pool(name="psz", bufs=4, space="PSUM") as pszp,
        ):
            tinyb = constp.tile([128, 1], f32)
            nc.vector.memset(tinyb[:], TINY)
            shiftb = constp.tile([128, 1], f32)
            nc.vector.memset(shiftb[:], -SHIFT)
            epsb = constp.tile([128, 1], f32)
            nc.vector.memset(epsb[:], EPS)
            ones = constp.tile([128, 1], f32)
            nc.vector.memset(ones[:], 1.0)
            # preload all SWDGE indices + dstrel (partition-major layouts);
            # sliced so early chunks' indices land before the whole preload.
            isrc_sb = constp.tile([128, t_total, 8], i16)
            qt = -(-t_total // 4)
            for _q in range(4):
                lo_t, hi_t = _q * qt, min((_q + 1) * qt, t_total)
                nc.sync.dma_start(
                    isrc_sb[:, lo_t:hi_t, :], isrc_d[:, lo_t:hi_t, :]
                )
            drel_sb = constp.tile([128, t_total], bf16)
            nc.sync.dma_start(drel_sb[:], drel_d[:, :])

            # iotat_rep[p, t, n] = n (host constant)
            iotat_rep = constp.tile([128, CHUNK_TILES, 128], bf16)
            nc.sync.dma_start(iotat_rep[:], iotat_rep_d[:, :, :])
            # niotac[p, 0] = -p, bias for the Scalar-engine ohT build
            niotac = constp.tile([128, 1], f32)
            nc.sync.dma_start(niotac[:], niota_d.ap().unsqueeze(1))
            onesb = constp.tile([128, 1], f32)
            nc.vector.memset(onesb[:], 1.0)

            h_all = finp.tile([128, D, nw], f32)
            # running BatchNorm stats, accumulated per phase
            s1acc = finp.tile([128, D], f32, tag="s1a")
            nc.vector.memset(s1acc[:], 0.0)
            s2acc = finp.tile([128, D], f32, tag="s2a")
            nc.vector.memset(s2acc[:], 0.0)

            g = 0  # global tile cursor
            kq = 0  # chunk counter for SWDGE queue rotation
            for ph in plan["phases"]:
                nwp = len(ph)
                w0 = ph[0]
                psb = psump.tile([128, MAX_PSUM_WIN, 128], f32, tag="psb")
                zwin = datap.tile([128, MAX_PSUM_WIN, D], bf16, tag="zwin")
                nc.sync.dma_start(
                    zwin[:, 0:nwp, :],
                    zs_d[w0 * WIN : (w0 + nwp) * WIN, :].rearrange(
                        "(w p) d -> p w d", p=128
                    ),
                )
                ph_secs = [
                    (ty, tl) for (ty, tl) in plan["sections"] if tl[0][0] in ph
                ]
                for ty, tl in ph_secs:
                    table = z_d[0:lo_max, :] if ty == 0 else z_d[split:n_nodes, :]
                    for c0 in range(0, len(tl), CT):
                        ct = min(CT, len(tl) - c0)
                        t0 = g + c0
                        ne = ct * 128

                        zsrc = datap.tile([128, CT, D], f32, tag="zsrc")
                        for g0 in range(0, ct, 8):
                            gc = min(8, ct - g0)
                            nc.gpsimd.dma_gather(
                                zsrc[:, g0 : g0 + gc, :],
                                table,
                                isrc_sb[:, t0 + g0 : t0 + g0 + gc, :],
                                gc * 128,
                                gc * 128,
                                D,
                                queue_num=kq % 4,
                            )
                            kq += 1

                        # node-major one-hot: ohT[n, e] = (drel_e == n).
                        # A/B by chunk parity: even -> Scalar square+relu,
                        # odd -> DVE is_equal with both operands contiguous
                        # (tests whether the 2x DVE mode engages).
                        drbc = ohp.tile([128, CT * 128], bf16, tag="drbc")
                        nc.sync.dma_start(
                            drbc[:, 0:ne],
                            drelf_d[t0 * 128 : t0 * 128 + ne].partition_broadcast(
                                128
                            ),
                        )
                        ohT = ohp.tile([128, CT * 128], bf16, tag="ohT")
                        nc.scalar.activation(
                            ohT[:, 0:ne], drbc[:, 0:ne], ACTF.Square,
                            bias=niotac[:], scale=1.0,
                        )
                        nc.scalar.activation(
                            ohT[:, 0:ne], ohT[:, 0:ne], ACTF.Relu,
                            bias=onesb[:], scale=-1.0,
                        )

                        # z[dst] expansion: psum_zd[:, t, :] = ohT_t^T @ zwin_t
                        # (pzd spans 2 PSUM banks at CT=16; a start=True
                        # matmul clears its whole bank, so open/close one
                        # accumulation group per 8-tile bank)
                        pzd = pszp.tile([128, CT, D], f32, tag="zd")
                        for tl_i in range(ct):
                            win = tile_meta[t0 + tl_i][0]
                            nc.tensor.matmul(
                                pzd[:, tl_i, :],
                                ohT[:, tl_i * 128 : (tl_i + 1) * 128],
                                zwin[:, win - w0, :],
                                start=tl_i % 8 == 0,
                                stop=tl_i % 8 == 7 or tl_i == ct - 1,
                            )

                        # edge scores and weights (prod in bf16 so the
                        # reduce reads 2 elems/cycle)
                        prod = datap.tile([128, CT, D], bf16, tag="prod")
                        e = smallp.tile([128, CT], f32, tag="e")
                        wt = smallp.tile([128, CT], f32, tag="wt")
                        nc.vector.tensor_mul(
                            prod[:, 0:ct, :], zsrc[:, 0:ct, :], pzd[:, 0:ct, :]
                        )
                        nc.vector.tensor_reduce(
                            e[:, 0:ct], prod[:, 0:ct, :], axis=AX.X, op=ALU.add
                        )
                        nc.scalar.activation(
                            e[:, 0:ct], e[:, 0:ct], ACTF.Relu,
                            bias=0.0, scale=1.0,
                        )
                        nc.scalar.activation(
                            wt[:, 0:ct], e[:, 0:ct], ACTF.Exp,
                            bias=shiftb[:], scale=1.0,
                        )

                        # vals = [w * z_src | w] in bf16
                        vals = datap.tile([128, CT, D + 1], bf16, tag="vals")
                        nc.scalar.copy(vals[:, 0:ct, D], wt[:, 0:ct])
                        nc.vector.tensor_mul(
                            vals[:, 0:ct, 0:D],
                            zsrc[:, 0:ct, :],
                            wt[:, 0:ct].unsqueeze(2).broadcast_to((128, ct, D)),
                        )

                        # aggregation one-hot (edge-major) in bf16
                        oh = ohp.tile([128, CT, 128], bf16, tag="oh")
                        nc.vector.tensor_tensor(
                            oh[:, 0:ct, :],
                            iotat_rep[:, 0:ct, :],
                            drel_sb[:, t0 : t0 + ct]
                            .unsqueeze(2)
                            .broadcast_to((128, ct, 128)),
                            op=ALU.is_equal,
                        )

                        for tl_i in range(ct):
                            win, st, sp = tile_meta[t0 + tl_i]
                            slot = win - w0
                            nc.tensor.matmul(
                                psb[:, slot, 0 : D + 1],
                                oh[:, tl_i, :],
                                vals[:, tl_i, :],
                                start=st,
                                stop=sp,
                            )
                    g += len(tl)

                # drain phase: h = num / denom, written feature-major
                denp = smallp.tile([128, MAX_PSUM_WIN], f32, tag="den")
                recp = smallp.tile([128, MAX_PSUM_WIN], f32, tag="rec")
                nc.scalar.activation(
                    denp[:, 0:nwp], psb[:, 0:nwp, D], ACTF.Identity,
                    bias=tinyb[:], scale=1.0,
                )
                nc.vector.reciprocal(recp[:, 0:nwp], denp[:, 0:nwp])
                nc.vector.tensor_mul(
                    h_all[:, :, w0 : w0 + nwp].transpose((0, 2, 1)),
                    psb[:, 0:nwp, 0:D],
                    recp[:, 0:nwp].unsqueeze(2).broadcast_to((128, nwp, D)),
                )

                # accumulate BatchNorm partial stats for this phase so the
                # AllReduce can start right after the last drain
                hsqp = smallp.tile([128, D, MAX_PSUM_WIN], f32, tag="hsqp")
                nc.scalar.square(
                    hsqp[:, :, 0:nwp], h_all[:, :, w0 : w0 + nwp]
                )
                s1t = smallp.tile([128, D], f32, tag="s1t")
                s2t = smallp.tile([128, D], f32, tag="s2t")
                nc.vector.tensor_reduce(
                    s1t[:], h_all[:, :, w0 : w0 + nwp], axis=AX.X, op=ALU.add
                )
                nc.vector.tensor_add(s1acc[:], s1acc[:], s1t[:])
                nc.vector.tensor_reduce(
                    s2t[:], hsqp[:, :, 0:nwp], axis=AX.X, op=ALU.add
                )
                nc.vector.tensor_add(s2acc[:], s2acc[:], s2t[:])

            # ---- BatchNorm stats: s1 = sum(h), s2 = sum(h^2) over all nodes
            stats = smallp.tile([128, 2 * D], f32, tag="stats")
            nc.scalar.copy(stats[:, 0:D], s1acc[:])
            nc.scalar.copy(stats[:, D : 2 * D], s2acc[:])

            ps = pszp.tile([1, 2 * D], f32, tag="zd")
            nc.tensor.matmul(ps[:], ones[:], stats[:], start=True, stop=True)
            srow = smallp.tile([1, 2 * D], f32, tag="srow")
            nc.scalar.copy(srow[:], ps[:])

            cc_in = dramp.tile([1, 2 * D], f32)
            cc_out = dramp.tile([1, 2 * D], f32)
            nc.sync.dma_start(cc_in[:], srow[:])
            nc.gpsimd.collective_compute(
                "AllReduce",
                ALU.add,
                ins=[cc_in.opt()],
                outs=[cc_out.opt()],
                replica_groups=[list(range(NCORES))],
            )

            G = smallp.tile([128, 2 * D], f32, tag="G")
            nc.sync.dma_start(G[:], cc_out[:].squeeze(0).partition_broadcast(128))
            gbB = constp.tile([128, 2 * D], f32)
            nc.sync.dma_start(gbB[:], gb_d.ap().flatten().partition_broadcast(128))

            inv_n = 1.0 / float(n_total_nodes)
            mean = smallp.tile([128, D], f32, tag="mean")
            var = smallp.tile([128, D], f32, tag="var")
            nc.scalar.mul(mean[:], G[:, 0:D], inv_n)
            nc.scalar.mul(var[:], G[:, D : 2 * D], inv_n)
            msq = smallp.tile([128, D], f32, tag="msq")
            nc.vector.tensor_mul(msq[:], mean[:], mean[:])
            nc.vector.tensor_sub(var[:], var[:], msq[:])
            std = smallp.tile([128, D], f32, tag="std")
            nc.scalar.activation(std[:], var[:], ACTF.Sqrt, bias=epsb[:], scale=1.0)
            rstd = smallp.tile([128, D], f32, tag="rstd")
            nc.vector.reciprocal(rstd[:], std[:])

            a = smallp.tile([128, D], f32, tag="a")
            b = smallp.tile([128, D], f32, tag="b")
            nc.vector.tensor_mul(a[:], gbB[:, 0:D], rstd[:])
            nc.vector.tensor_mul(b[:], mean[:], a[:])
            nc.vector.tensor_sub(b[:], gbB[:, D : 2 * D], b[:])

            # y stored node-major so the output DMA gets 256B-contiguous
            # runs; built and stored in window blocks so the output DMA
            # overlaps the BN math of later blocks.
            outv = out_d.ap().rearrange("(c p) f -> p c f", p=128)
            nblk = 6
            bw = -(-nw // nblk)
            for blk in range(nblk):
                wl, wh = blk * bw, min((blk + 1) * bw, nw)
                if wl >= wh:
                    break
                y = yblkp.tile([128, bw, D], f32, tag="y")
                nc.vector.tensor_mul(
                    y[:, 0 : wh - wl, :],
                    h_all[:, :, wl:wh].transpose((0, 2, 1)),
                    a[:].unsqueeze(1).broadcast_to((128, wh - wl, D)),
                )
                nc.vector.tensor_add(
                    y[:, 0 : wh - wl, :],
                    y[:, 0 : wh - wl, :],
                    b[:].unsqueeze(1).broadcast_to((128, wh - wl, D)),
                )
                nc.vector.tensor_relu(
                    y[:, 0 : wh - wl, :], y[:, 0 : wh - wl, :]
                )
                nc.sync.dma_start(outv[:, wl:wh, :], y[:, 0 : wh - wl, :])

    nc.compile()
    return nc


# ---------------------------------------------------------------- entry point
TRACE = False          # set True by test harnesses to capture exec_time_ns
LAST_RESULT = None     # BassKernelResults of the most recent kernel() call


def kernel(**inputs):
    z = inputs["z"]
    src = inputs["src"]
    dst = inputs["dst"]
    gamma = inputs["gamma"]
    beta = inputs["beta"]

    from concourse.bass_utils import run_bass_kernel_spmd

    in_maps, plan = prep_inputs(z, src, dst, gamma, beta)
    nc = build_nc(plan)
    res = run_bass_kernel_spmd(
        nc, in_maps, core_ids=list(range(NCORES)), trace=TRACE
    )
    global LAST_RESULT
    LAST_RESULT = res

    npc = CFG["npc"]
    out = np.empty((N_NODES, D), dtype=np.float32)
    for c in range(NCORES):
        out[c * npc : (c + 1) * npc] = res.results[c]["out"][:npc]
    return out
